# revision 24
# baseline (speedup 1.0000x reference)
"""FMoELinear grouped GEMM on 8 Trainium2 NeuronCores (expert parallelism).

Strategy
--------
Tokens arrive pre-grouped contiguously by expert, and the per-expert counts
are host-visible routing metadata.  All routing therefore happens on the
host: the 64 expert segments are split into 8*G near-equal "pieces"
(arbitrary token offsets, optimized by local search so that the 8 pieces
sharing a slot rank have near-equal tile counts), dealt onto the 8 cores
rank-matched so that slot g has the same tile count K[g] on every core.
That makes one SPMD Bass program valid for all 8 cores with ~4% padding.

Per core the device kernel computes, slot by slot:
    out[tile] = sum_k xT[k,tile].T @ wT[k] + bias      (psum accumulation)
with x shipped host-transposed ([128 in-feat partitions, token columns]) so
the PE needs no on-chip transposes, and the output written in
[128, tiles, 512] layout (partition = token-within-tile) which the host
untransposes during the gather.

Numerics: MODE selects the matmul path.
  "bf16"  - bf16 matmuls, fp16 output (~2e-3 rel err, halves DMA bytes)
  "f32"   - native fp32 matmuls (4 cycles/row on PE)
  "f32r"  - hardware round-to-~13-bit fast fp32 (1 cycle/row), ~1e-4 rel err
"""
import sys
sys.path.insert(0, "/opt/trn_rl_repo")

import numpy as np
import ml_dtypes

import concourse.bass as bass
import concourse.tile as tile
from concourse import bacc, mybir
from concourse.bass_utils import run_bass_kernel_spmd

# If the environment requests NTFF tracing (BASS_TRACE=1) but this image's
# antenv lacks the axon profiling hook module, run_bass_kernel_spmd would
# crash on import. Register a null hook so it degrades to trace-skipped.
try:
    from antenv.axon_hooks import get_axon_ntff_profile_hook as _hook_probe  # noqa: F401
except ImportError:
    import types as _types

    import antenv as _antenv

    _mod = _types.ModuleType("antenv.axon_hooks")
    _mod.get_axon_ntff_profile_hook = lambda: None
    _mod.set_axon_ntff_profile_hook = lambda h: None
    sys.modules.setdefault("antenv.axon_hooks", _mod)
    _antenv.axon_hooks = sys.modules["antenv.axon_hooks"]

F32 = mybir.dt.float32
F32R = mybir.dt.float32r
BF16 = mybir.dt.bfloat16
F16 = mybir.dt.float16

NUM_EXPERT = 64
IN_FEAT = 512
OUT_FEAT = 512
P = 128
KT = IN_FEAT // P          # 4 contraction k-tiles
NCORES = 8
G_SLOTS = 11               # slots (pieces) per core
CX = 12                    # token tiles per x-DMA chunk
XBUFS = 6
OBUFS = 4
WBUFS = 3

MODE = "bf16"              # "bf16" | "f32" | "f32r"
TRACE = False              # set True (e.g. from test.py) to profile
LAST_RESULT = None         # BassKernelResults of the last run

_program_cache = {}
_sched_cache = {}


# ----------------------------------------------------------------- schedule
def _schedule(counts):
    """Split experts into 8*G pieces (arbitrary offsets), octets rank-matched.

    Returns (K, slots): K[g] = tile count of slot g (same on all cores);
    slots[core][g] = (expert, row_start, nrows) with nrows <= K[g]*128.
    """
    key = tuple(int(c) for c in counts)
    if key in _sched_cache:
        return _sched_cache[key]
    import heapq

    counts = [int(c) for c in counts]
    starts = np.concatenate([[0], np.cumsum(counts)]).astype(np.int64)
    live = [e for e in range(len(counts)) if counts[e] > 0]
    M = NCORES * G_SLOTS

    # LPT: split each expert into near-equal pieces, largest first
    p = {e: 1 for e in live}
    h = [(-counts[e], e) for e in live]
    heapq.heapify(h)
    while sum(p.values()) < min(M, sum(counts[e] for e in live)):
        sz, e = heapq.heappop(h)
        if p[e] >= counts[e]:
            continue
        p[e] += 1
        heapq.heappush(h, (-counts[e] / p[e], e))
    pieces = []  # [size, expert]
    for e in live:
        n = p[e]
        base, rem = divmod(counts[e], n)
        for i in range(n):
            pieces.append([base + (1 if i < rem else 0), e])
    while len(pieces) < M:
        pieces.append([0, -1])

    def T_of(ps):
        s = sorted(ps, key=lambda t: -t[0])
        return sum((s[g * NCORES][0] + P - 1) // P for g in range(G_SLOTS))

    # targeted descent: shed each octet-max piece's overhang onto siblings
    def targeted(ps):
        best = T_of(ps)
        for _ in range(60):
            order = sorted(range(len(ps)), key=lambda i: -ps[i][0])
            octet_K = {}
            for g in range(G_SLOTS):
                Kg = (ps[order[g * NCORES]][0] + P - 1) // P
                for i in order[g * NCORES:(g + 1) * NCORES]:
                    octet_K[i] = Kg
            improved = False
            for g in range(G_SLOTS):
                i = order[g * NCORES]
                sz, e = ps[i]
                if e < 0 or sz == 0:
                    continue
                overh = sz - ((sz + P - 1) // P - 1) * P
                room = sorted(
                    ((octet_K[j] * P - ps[j][0], j)
                     for j in range(len(ps))
                     if ps[j][1] == e and j != i),
                    reverse=True)
                need, moves = overh, []
                for rm, j in room:
                    if need <= 0:
                        break
                    take = min(rm, need)
                    if take > 0:
                        moves.append((j, take))
                        need -= take
                if need <= 0 and moves:
                    undo = [(j, ps[j][0]) for j, _ in moves] + [(i, sz)]
                    for j, t in moves:
                        ps[j][0] += t
                    ps[i][0] -= overh
                    nT = T_of(ps)
                    if nT < best:
                        best = nT
                        improved = True
                    else:
                        for j, v in undo:
                            ps[j][0] = v
            if not improved:
                break
        return best

    # local search: shift tokens between same-expert sibling pieces
    rng = np.random.default_rng(1)
    targeted(pieces)
    best = T_of(pieces)
    sib_idx = {}
    for j, (_, e) in enumerate(pieces):
        sib_idx.setdefault(e, []).append(j)
    for _ in range(60000):
        i = int(rng.integers(len(pieces)))
        e = pieces[i][1]
        if e < 0 or len(sib_idx[e]) < 2:
            continue
        sibs = sib_idx[e]
        j = sibs[int(rng.integers(len(sibs)))]
        if j == i:
            continue
        amt = int(rng.integers(1, 257))
        if pieces[i][0] <= amt:
            continue
        pieces[i][0] -= amt
        pieces[j][0] += amt
        c = T_of(pieces)
        if c <= best:
            best = c
        else:
            pieces[i][0] += amt
            pieces[j][0] -= amt

    # assign offsets within each expert in piece order
    off_in_e = {e: 0 for e in live}
    recs = []  # (size, expert, row_start)
    for sz, e in pieces:
        if e < 0 or sz == 0:
            recs.append((0, 0, 0))
        else:
            recs.append((sz, e, int(starts[e]) + off_in_e[e]))
            off_in_e[e] += sz
    recs.sort(key=lambda t: -t[0])

    K = []
    slots = [[] for _ in range(NCORES)]
    for g in range(G_SLOTS):
        octet = recs[g * NCORES:(g + 1) * NCORES]
        K.append((octet[0][0] + P - 1) // P)
        for i, (sz, e, r0) in enumerate(octet):
            slots[i].append((e, int(r0), int(sz)))
    while K and K[-1] == 0:
        K.pop()
        for s in slots:
            s.pop()
    _sched_cache[key] = (K, slots)
    return K, slots


def _chunks(kg, first_slot):
    """Chunk sizes for a slot of kg tiles. Slot 0 ramps up (small first
    chunk so the first matmul starts early)."""
    sizes = []
    rem = kg
    if first_slot:
        for c in (2, 6, 10):
            if rem <= 0:
                break
            c = min(c, rem)
            sizes.append(c)
            rem -= c
    while rem > 0:
        c = min(CX, rem)
        sizes.append(c)
        rem -= c
    return sizes


# ------------------------------------------------------------ device program
def _build_program(K, mode, bias_zero):
    G = len(K)
    T = sum(K)
    nc = bacc.Bacc(None)

    mmdt = {"f32r": F32R, "f32": F32, "bf16": BF16}[mode]
    outdt = F16 if mode == "bf16" else F32
    xt_d = nc.declare_dram_parameter("xt", [P, KT, T * P], mmdt, isOutput=False)
    wt_d = nc.declare_dram_parameter("wt", [G, P, KT, OUT_FEAT], mmdt, isOutput=False)
    if not bias_zero:
        b_d = nc.declare_dram_parameter("bias", [G, 1, OUT_FEAT], F32, isOutput=False)
    out_d = nc.declare_dram_parameter("out", [P, T, OUT_FEAT], outdt, isOutput=True)

    with tile.TileContext(nc) as tc:
        with (
            tc.tile_pool(name="w", bufs=WBUFS) as wp,
            tc.tile_pool(name="x", bufs=XBUFS) as xp,
            tc.tile_pool(name="b", bufs=2) as bp,
            tc.tile_pool(name="o", bufs=OBUFS) as op,
            tc.tile_pool(name="ps", bufs=8, space=bass.MemorySpace.PSUM) as pp,
        ):
            # PE warm-up: matmuls on scratch data at t~0 so the HAM
            # clock-gate reaches 8/8 (2.4 GHz) before the first real matmul
            # (which otherwise runs its first ~3.4us at 1.2 GHz). Reuses the
            # x and psum buffer rings (first allocation each) - no footprint.
            warm_sb = xp.tile([P, KT, CX * P], mmdt, tag="x")
            nc.vector.memset(warm_sb[:, 0, :384], 0.0)
            ps = pp.tile([P, OUT_FEAT], F32, tag="ps")
            for _ in range(36):
                nc.tensor.matmul(ps[:, :256], warm_sb[:, 0, :P],
                                 warm_sb[:, 0, 128:384],
                                 start=True, stop=True)

            off = 0
            ncopy = 0
            nstore = 0
            for g in range(G):
                kg = K[g]
                # slot 0's weights go on the sync queue, between the first
                # (small) x chunk and the rest, so the first matmuls start as
                # soon as those two transfers land; later slots prefetch on
                # the gpsimd queue
                w_sb = wp.tile([P, KT, OUT_FEAT], mmdt, tag="w")
                if g != 0:
                    nc.gpsimd.dma_start(w_sb[:], wt_d[g])
                if not bias_zero:
                    b1_sb = bp.tile([1, OUT_FEAT], F32, tag="b1")
                    (nc.scalar if g == 0 else nc.gpsimd).dma_start(
                        b1_sb[:], b_d[g])
                    b_sb = bp.tile([P, OUT_FEAT], F32, tag="b")
                    nc.gpsimd.partition_broadcast(b_sb[:], b1_sb[:])

                c0 = 0
                chunk_list = _chunks(kg, g == 0)
                for ci, cw in enumerate(chunk_list):
                    w_cols = cw * P
                    col0 = (off + c0) * P
                    x_sb = xp.tile([P, KT, CX * P], mmdt, tag="x")
                    nc.sync.dma_start(
                        x_sb[:, :, :w_cols], xt_d[:, :, col0:col0 + w_cols])
                    if g == 0 and ci == 0:
                        for k in range(KT):
                            nc.sync.dma_start(w_sb[:, k, :], wt_d[g, :, k, :])
                    o_sb = op.tile([P, CX, OUT_FEAT], outdt)
                    last_chunk = (g == G - 1 and ci == len(chunk_list) - 1)
                    step = 4 if last_chunk else (cw + 1) // 2
                    prev_s = 0
                    for t in range(cw):
                        ps = pp.tile([P, OUT_FEAT], F32, tag="ps")
                        for k in range(KT):
                            nc.tensor.matmul(
                                ps[:], x_sb[:, k, t * P:(t + 1) * P],
                                w_sb[:, k, :],
                                start=(k == 0), stop=(k == KT - 1))
                        # psum -> sbuf, rotating engines to keep any single
                        # one off the critical path
                        if bias_zero:
                            if ncopy % 3 != 2:
                                nc.vector.tensor_copy(o_sb[:, t, :], ps[:])
                            else:
                                nc.scalar.copy(o_sb[:, t, :], ps[:])
                        else:
                            nc.vector.tensor_add(o_sb[:, t, :], ps[:], b_sb[:])
                        ncopy += 1
                        # store slices as soon as their copies are done,
                        # alternating hw queues so neither stub stream nor
                        # queue becomes the straggler
                        if t - prev_s + 1 >= step or t == cw - 1:
                            nc.scalar.dma_start(
                                out_d[:, off + c0 + prev_s:off + c0 + t + 1, :],
                                o_sb[:, prev_s:t + 1, :])
                            prev_s = t + 1
                    c0 += cw
                off += kg
    nc.compile()
    return nc


# ------------------------------------------------------------------- kernel
def kernel(inp, fwd_expert_count, weight, bias):
    inp = np.asarray(inp, dtype=np.float32)
    weight = np.asarray(weight, dtype=np.float32)
    bias = np.asarray(bias, dtype=np.float32)
    counts = np.asarray(fwd_expert_count)

    K, slots = _schedule(counts)
    G, T = len(K), sum(K)
    off = np.concatenate([[0], np.cumsum(K)]).astype(np.int64)

    bias_zero = not bool(np.any(bias))
    key = (tuple(K), MODE, bias_zero)
    if key not in _program_cache:
        _program_cache[key] = _build_program(K, MODE, bias_zero)
    nc = _program_cache[key]

    npdt = ml_dtypes.bfloat16 if MODE == "bf16" else np.float32

    # per-expert transposed weights [P, KT, OUT]: wT[p, k, o] = weight[e][o, 128k+p]
    wT = {}
    for e in set(e for s in slots for (e, _, n) in s if n > 0):
        wT[e] = np.ascontiguousarray(
            weight[e].T.reshape(KT, P, OUT_FEAT).transpose(1, 0, 2)).astype(npdt)

    in_maps = []
    for core in range(NCORES):
        xt = np.zeros((P, KT, T * P), dtype=npdt)
        wt = np.zeros((G, P, KT, OUT_FEAT), dtype=npdt)
        brep = np.zeros((G, 1, OUT_FEAT), dtype=np.float32)
        for g, (e, r0, n) in enumerate(slots[core]):
            if n > 0:
                blk = inp[r0:r0 + n].T.reshape(KT, P, n).transpose(1, 0, 2)
                col0 = int(off[g]) * P
                xt[:, :, col0:col0 + n] = blk.astype(npdt)
                wt[g] = wT[e]
                brep[g, 0] = bias[e]
        m = {"xt": xt, "wt": wt}
        if not bias_zero:
            m["bias"] = brep
        in_maps.append(m)

    global LAST_RESULT
    res = run_bass_kernel_spmd(
        nc, in_maps, list(range(NCORES)),
        trace=TRACE, trace_cores=list(range(NCORES)) if TRACE else None,
        stitch_traces=False)
    LAST_RESULT = res

    out = np.empty((int(np.sum(np.asarray(counts, dtype=np.int64))), OUT_FEAT),
                   dtype=np.float32)
    for core in range(NCORES):
        arr = res.results[core]["out"]  # [P, T, OUT]
        for g, (e, r0, n) in enumerate(slots[core]):
            if n > 0:
                o0 = int(off[g])
                kg = K[g]
                blk = np.asarray(arr[:, o0:o0 + kg, :], dtype=np.float32)
                blk = blk.transpose(1, 0, 2).reshape(kg * P, OUT_FEAT)
                out[r0:r0 + n] = blk[:n]
    return out


# revision 25
# speedup vs baseline: 1.1522x; 1.1522x over previous
"""FMoELinear grouped GEMM on 8 Trainium2 NeuronCores (expert parallelism).

Strategy
--------
Tokens arrive pre-grouped contiguously by expert, and the per-expert counts
are host-visible routing metadata.  All routing therefore happens on the
host: the 64 expert segments are split into 8*G near-equal "pieces"
(arbitrary token offsets, optimized by local search so that the 8 pieces
sharing a slot rank have near-equal tile counts), dealt onto the 8 cores
rank-matched so that slot g has the same tile count K[g] on every core.
That makes one SPMD Bass program valid for all 8 cores with ~4% padding.

Per core the device kernel computes, slot by slot:
    out[tile] = sum_k xT[k,tile].T @ wT[k] + bias      (psum accumulation)
with x shipped host-transposed ([128 in-feat partitions, token columns]) so
the PE needs no on-chip transposes, and the output written in
[128, tiles, 512] layout (partition = token-within-tile) which the host
untransposes during the gather.

Numerics: MODE selects the matmul path.
  "bf16"  - bf16 matmuls, fp16 output (~2e-3 rel err, halves DMA bytes)
  "f32"   - native fp32 matmuls (4 cycles/row on PE)
  "f32r"  - hardware round-to-~13-bit fast fp32 (1 cycle/row), ~1e-4 rel err
"""
import sys
sys.path.insert(0, "/opt/trn_rl_repo")

import numpy as np
import ml_dtypes

import concourse.bass as bass
import concourse.tile as tile
from concourse import bacc, mybir
from concourse.bass_utils import run_bass_kernel_spmd

# If the environment requests NTFF tracing (BASS_TRACE=1) but this image's
# antenv lacks the axon profiling hook module, run_bass_kernel_spmd would
# crash on import. Register a null hook so it degrades to trace-skipped.
try:
    from antenv.axon_hooks import get_axon_ntff_profile_hook as _hook_probe  # noqa: F401
except ImportError:
    import types as _types

    import antenv as _antenv

    _mod = _types.ModuleType("antenv.axon_hooks")
    _mod.get_axon_ntff_profile_hook = lambda: None
    _mod.set_axon_ntff_profile_hook = lambda h: None
    sys.modules.setdefault("antenv.axon_hooks", _mod)
    _antenv.axon_hooks = sys.modules["antenv.axon_hooks"]

F32 = mybir.dt.float32
F32R = mybir.dt.float32r
BF16 = mybir.dt.bfloat16
F16 = mybir.dt.float16

NUM_EXPERT = 64
IN_FEAT = 512
OUT_FEAT = 512
P = 128
KT = IN_FEAT // P          # 4 contraction k-tiles
NCORES = 8
G_SLOTS = 11               # slots (pieces) per core
CX = 12                    # token tiles per x-DMA chunk
XBUFS = 6
OBUFS = 4
WBUFS = 3

MODE = "bf16"              # "bf16" | "f32" | "f32r"
TRACE = False              # set True (e.g. from test.py) to profile
LAST_RESULT = None         # BassKernelResults of the last run

_program_cache = {}
_sched_cache = {}


# ----------------------------------------------------------------- schedule
def _schedule(counts):
    """Split experts into 8*G pieces (arbitrary offsets), octets rank-matched.

    Returns (K, slots): K[g] = tile count of slot g (same on all cores);
    slots[core][g] = (expert, row_start, nrows) with nrows <= K[g]*128.
    """
    key = tuple(int(c) for c in counts)
    if key in _sched_cache:
        return _sched_cache[key]
    import heapq

    counts = [int(c) for c in counts]
    starts = np.concatenate([[0], np.cumsum(counts)]).astype(np.int64)
    live = [e for e in range(len(counts)) if counts[e] > 0]
    M = NCORES * G_SLOTS

    # LPT: split each expert into near-equal pieces, largest first
    p = {e: 1 for e in live}
    h = [(-counts[e], e) for e in live]
    heapq.heapify(h)
    while sum(p.values()) < min(M, sum(counts[e] for e in live)):
        sz, e = heapq.heappop(h)
        if p[e] >= counts[e]:
            continue
        p[e] += 1
        heapq.heappush(h, (-counts[e] / p[e], e))
    pieces = []  # [size, expert]
    for e in live:
        n = p[e]
        base, rem = divmod(counts[e], n)
        for i in range(n):
            pieces.append([base + (1 if i < rem else 0), e])
    while len(pieces) < M:
        pieces.append([0, -1])

    def T_of(ps):
        s = sorted(ps, key=lambda t: -t[0])
        return sum((s[g * NCORES][0] + P - 1) // P for g in range(G_SLOTS))

    # targeted descent: shed each octet-max piece's overhang onto siblings
    def targeted(ps):
        best = T_of(ps)
        for _ in range(60):
            order = sorted(range(len(ps)), key=lambda i: -ps[i][0])
            octet_K = {}
            for g in range(G_SLOTS):
                Kg = (ps[order[g * NCORES]][0] + P - 1) // P
                for i in order[g * NCORES:(g + 1) * NCORES]:
                    octet_K[i] = Kg
            improved = False
            for g in range(G_SLOTS):
                i = order[g * NCORES]
                sz, e = ps[i]
                if e < 0 or sz == 0:
                    continue
                overh = sz - ((sz + P - 1) // P - 1) * P
                room = sorted(
                    ((octet_K[j] * P - ps[j][0], j)
                     for j in range(len(ps))
                     if ps[j][1] == e and j != i),
                    reverse=True)
                need, moves = overh, []
                for rm, j in room:
                    if need <= 0:
                        break
                    take = min(rm, need)
                    if take > 0:
                        moves.append((j, take))
                        need -= take
                if need <= 0 and moves:
                    undo = [(j, ps[j][0]) for j, _ in moves] + [(i, sz)]
                    for j, t in moves:
                        ps[j][0] += t
                    ps[i][0] -= overh
                    nT = T_of(ps)
                    if nT < best:
                        best = nT
                        improved = True
                    else:
                        for j, v in undo:
                            ps[j][0] = v
            if not improved:
                break
        return best

    # local search: shift tokens between same-expert sibling pieces
    rng = np.random.default_rng(1)
    targeted(pieces)
    best = T_of(pieces)
    sib_idx = {}
    for j, (_, e) in enumerate(pieces):
        sib_idx.setdefault(e, []).append(j)
    for _ in range(120000):
        i = int(rng.integers(len(pieces)))
        e = pieces[i][1]
        if e < 0 or len(sib_idx[e]) < 2:
            continue
        sibs = sib_idx[e]
        j = sibs[int(rng.integers(len(sibs)))]
        if j == i:
            continue
        amt = int(rng.integers(1, 257))
        if pieces[i][0] <= amt:
            continue
        pieces[i][0] -= amt
        pieces[j][0] += amt
        c = T_of(pieces)
        if c <= best:
            best = c
        else:
            pieces[i][0] += amt
            pieces[j][0] -= amt

    # assign offsets within each expert in piece order
    off_in_e = {e: 0 for e in live}
    recs = []  # (size, expert, row_start)
    for sz, e in pieces:
        if e < 0 or sz == 0:
            recs.append((0, 0, 0))
        else:
            recs.append((sz, e, int(starts[e]) + off_in_e[e]))
            off_in_e[e] += sz
    recs.sort(key=lambda t: -t[0])

    K = []
    slots = [[] for _ in range(NCORES)]
    for g in range(G_SLOTS):
        octet = recs[g * NCORES:(g + 1) * NCORES]
        K.append((octet[0][0] + P - 1) // P)
        for i, (sz, e, r0) in enumerate(octet):
            slots[i].append((e, int(r0), int(sz)))
    while K and K[-1] == 0:
        K.pop()
        for s in slots:
            s.pop()
    _sched_cache[key] = (K, slots)
    return K, slots


def _chunks(kg, first_slot):
    """Chunk sizes for a slot of kg tiles. Slot 0 ramps up (small first
    chunk so the first matmul starts early)."""
    sizes = []
    rem = kg
    if first_slot:
        for c in (2, 6, 10):
            if rem <= 0:
                break
            c = min(c, rem)
            sizes.append(c)
            rem -= c
    while rem > 0:
        c = min(CX, rem)
        sizes.append(c)
        rem -= c
    return sizes


# ------------------------------------------------------------ device program
def _build_program(K, mode, bias_zero):
    G = len(K)
    T = sum(K)
    nc = bacc.Bacc(None)

    mmdt = {"f32r": F32R, "f32": F32, "bf16": BF16}[mode]
    outdt = F16 if mode == "bf16" else F32
    xt_d = nc.declare_dram_parameter("xt", [P, KT, T * P], mmdt, isOutput=False)
    wt_d = nc.declare_dram_parameter("wt", [G, P, KT, OUT_FEAT], mmdt, isOutput=False)
    if not bias_zero:
        b_d = nc.declare_dram_parameter("bias", [G, 1, OUT_FEAT], F32, isOutput=False)
    out_d = nc.declare_dram_parameter("out", [P, T, OUT_FEAT], outdt, isOutput=True)

    with tile.TileContext(nc) as tc:
        with (
            tc.tile_pool(name="w", bufs=WBUFS) as wp,
            tc.tile_pool(name="x", bufs=XBUFS) as xp,
            tc.tile_pool(name="b", bufs=2) as bp,
            tc.tile_pool(name="o", bufs=OBUFS) as op,
            tc.tile_pool(name="ps", bufs=8, space=bass.MemorySpace.PSUM) as pp,
        ):
            # PE warm-up: matmuls on scratch data at t~0 so the HAM
            # clock-gate reaches 8/8 (2.4 GHz) before the first real matmul
            # (which otherwise runs its first ~3.4us at 1.2 GHz). Reuses the
            # x and psum buffer rings (first allocation each) - no footprint.
            warm_sb = xp.tile([P, KT, CX * P], mmdt, tag="x")
            nc.vector.memset(warm_sb[:, 0, :384], 0.0)
            ps = pp.tile([P, OUT_FEAT], F32, tag="ps")
            for _ in range(36):
                nc.tensor.matmul(ps[:, :256], warm_sb[:, 0, :P],
                                 warm_sb[:, 0, 128:384],
                                 start=True, stop=True)

            off = 0
            ncopy = 0
            nstore = 0
            for g in range(G):
                kg = K[g]
                # slot 0's weights go on the sync queue, between the first
                # (small) x chunk and the rest, so the first matmuls start as
                # soon as those two transfers land; later slots prefetch on
                # the gpsimd queue
                w_sb = wp.tile([P, KT, OUT_FEAT], mmdt, tag="w")
                if g != 0:
                    nc.gpsimd.dma_start(w_sb[:], wt_d[g])
                if not bias_zero:
                    b1_sb = bp.tile([1, OUT_FEAT], F32, tag="b1")
                    (nc.scalar if g == 0 else nc.gpsimd).dma_start(
                        b1_sb[:], b_d[g])
                    b_sb = bp.tile([P, OUT_FEAT], F32, tag="b")
                    nc.gpsimd.partition_broadcast(b_sb[:], b1_sb[:])

                c0 = 0
                chunk_list = _chunks(kg, g == 0)
                for ci, cw in enumerate(chunk_list):
                    w_cols = cw * P
                    col0 = (off + c0) * P
                    x_sb = xp.tile([P, KT, CX * P], mmdt, tag="x")
                    nc.sync.dma_start(
                        x_sb[:, :, :w_cols], xt_d[:, :, col0:col0 + w_cols])
                    if g == 0 and ci == 0:
                        for k in range(KT):
                            nc.sync.dma_start(w_sb[:, k, :], wt_d[g, :, k, :])
                    o_sb = op.tile([P, CX, OUT_FEAT], outdt)
                    last_chunk = (g == G - 1 and ci == len(chunk_list) - 1)
                    step = 4 if last_chunk else (cw + 1) // 2
                    prev_s = 0
                    for t in range(cw):
                        ps = pp.tile([P, OUT_FEAT], F32, tag="ps")
                        for k in range(KT):
                            nc.tensor.matmul(
                                ps[:], x_sb[:, k, t * P:(t + 1) * P],
                                w_sb[:, k, :],
                                start=(k == 0), stop=(k == KT - 1))
                        # psum -> sbuf, rotating engines to keep any single
                        # one off the critical path
                        if bias_zero:
                            if ncopy % 3 != 2:
                                nc.vector.tensor_copy(o_sb[:, t, :], ps[:])
                            else:
                                nc.scalar.copy(o_sb[:, t, :], ps[:])
                        else:
                            nc.vector.tensor_add(o_sb[:, t, :], ps[:], b_sb[:])
                        ncopy += 1
                        # store slices as soon as their copies are done,
                        # alternating hw queues so neither stub stream nor
                        # queue becomes the straggler
                        if t - prev_s + 1 >= step or t == cw - 1:
                            nc.scalar.dma_start(
                                out_d[:, off + c0 + prev_s:off + c0 + t + 1, :],
                                o_sb[:, prev_s:t + 1, :])
                            prev_s = t + 1
                    c0 += cw
                off += kg
    nc.compile()
    return nc


# ------------------------------------------------------------------- kernel
def kernel(inp, fwd_expert_count, weight, bias):
    inp = np.asarray(inp, dtype=np.float32)
    weight = np.asarray(weight, dtype=np.float32)
    bias = np.asarray(bias, dtype=np.float32)
    counts = np.asarray(fwd_expert_count)

    K, slots = _schedule(counts)
    G, T = len(K), sum(K)
    off = np.concatenate([[0], np.cumsum(K)]).astype(np.int64)

    bias_zero = not bool(np.any(bias))
    key = (tuple(K), MODE, bias_zero)
    if key not in _program_cache:
        _program_cache[key] = _build_program(K, MODE, bias_zero)
    nc = _program_cache[key]

    npdt = ml_dtypes.bfloat16 if MODE == "bf16" else np.float32

    # per-expert transposed weights [P, KT, OUT]: wT[p, k, o] = weight[e][o, 128k+p]
    wT = {}
    for e in set(e for s in slots for (e, _, n) in s if n > 0):
        wT[e] = np.ascontiguousarray(
            weight[e].T.reshape(KT, P, OUT_FEAT).transpose(1, 0, 2)).astype(npdt)

    in_maps = []
    for core in range(NCORES):
        xt = np.zeros((P, KT, T * P), dtype=npdt)
        wt = np.zeros((G, P, KT, OUT_FEAT), dtype=npdt)
        brep = np.zeros((G, 1, OUT_FEAT), dtype=np.float32)
        for g, (e, r0, n) in enumerate(slots[core]):
            if n > 0:
                blk = inp[r0:r0 + n].T.reshape(KT, P, n).transpose(1, 0, 2)
                col0 = int(off[g]) * P
                xt[:, :, col0:col0 + n] = blk.astype(npdt)
                wt[g] = wT[e]
                brep[g, 0] = bias[e]
        m = {"xt": xt, "wt": wt}
        if not bias_zero:
            m["bias"] = brep
        in_maps.append(m)

    global LAST_RESULT
    res = run_bass_kernel_spmd(
        nc, in_maps, list(range(NCORES)),
        trace=TRACE, trace_cores=list(range(NCORES)) if TRACE else None,
        stitch_traces=False)
    LAST_RESULT = res

    out = np.empty((int(np.sum(np.asarray(counts, dtype=np.int64))), OUT_FEAT),
                   dtype=np.float32)
    for core in range(NCORES):
        arr = res.results[core]["out"]  # [P, T, OUT]
        for g, (e, r0, n) in enumerate(slots[core]):
            if n > 0:
                o0 = int(off[g])
                kg = K[g]
                blk = np.asarray(arr[:, o0:o0 + kg, :], dtype=np.float32)
                blk = blk.transpose(1, 0, 2).reshape(kg * P, OUT_FEAT)
                out[r0:r0 + n] = blk[:n]
    return out


# revision 30
# speedup vs baseline: 1.2088x; 1.0491x over previous
"""FMoELinear grouped GEMM on 8 Trainium2 NeuronCores (expert parallelism).

Strategy
--------
Tokens arrive pre-grouped contiguously by expert, and the per-expert counts
are host-visible routing metadata.  All routing therefore happens on the
host: the 64 expert segments are split into 8*G near-equal "pieces"
(arbitrary token offsets, optimized by local search so that the 8 pieces
sharing a slot rank have near-equal tile counts), dealt onto the 8 cores
rank-matched so that slot g has the same tile count K[g] on every core.
That makes one SPMD Bass program valid for all 8 cores with ~4% padding.

Per core the device kernel computes, slot by slot:
    out[tile] = sum_k xT[k,tile].T @ wT[k] + bias      (psum accumulation)
with x shipped host-transposed ([128 in-feat partitions, token columns]) so
the PE needs no on-chip transposes, and the output written in
[128, tiles, 512] layout (partition = token-within-tile) which the host
untransposes during the gather.

Numerics: MODE selects the matmul path.
  "bf16"  - bf16 matmuls, fp16 output (~2e-3 rel err, halves DMA bytes)
  "f32"   - native fp32 matmuls (4 cycles/row on PE)
  "f32r"  - hardware round-to-~13-bit fast fp32 (1 cycle/row), ~1e-4 rel err
"""
import sys
sys.path.insert(0, "/opt/trn_rl_repo")

import numpy as np
import ml_dtypes

import concourse.bass as bass
import concourse.tile as tile
from concourse import bacc, mybir
from concourse.bass_utils import run_bass_kernel_spmd

# If the environment requests NTFF tracing (BASS_TRACE=1) but this image's
# antenv lacks the axon profiling hook module, run_bass_kernel_spmd would
# crash on import. Register a null hook so it degrades to trace-skipped.
try:
    from antenv.axon_hooks import get_axon_ntff_profile_hook as _hook_probe  # noqa: F401
except ImportError:
    import types as _types

    import antenv as _antenv

    _mod = _types.ModuleType("antenv.axon_hooks")
    _mod.get_axon_ntff_profile_hook = lambda: None
    _mod.set_axon_ntff_profile_hook = lambda h: None
    sys.modules.setdefault("antenv.axon_hooks", _mod)
    _antenv.axon_hooks = sys.modules["antenv.axon_hooks"]

F32 = mybir.dt.float32
F32R = mybir.dt.float32r
BF16 = mybir.dt.bfloat16
F16 = mybir.dt.float16
F8E3 = mybir.dt.float8e3

# mode -> (x scale, w scale, mm dtype, numpy dtype); device output carries
# x_scale*w_scale times the true value, host divides it back out
_MODES = {
    "bf16": (1.0, 1.0),
    "f8": (2.0, 256.0),
}

NUM_EXPERT = 64
IN_FEAT = 512
OUT_FEAT = 512
P = 128
KT = IN_FEAT // P          # 4 contraction k-tiles
NCORES = 8
G_SLOTS = 11               # slots (pieces) per core
CX = 12                    # token tiles per x-DMA chunk
XBUFS = 6
OBUFS = 4
WBUFS = 3

MODE = "bf16"              # "bf16" | "f32" | "f32r"
TRACE = False              # set True (e.g. from test.py) to profile
LAST_RESULT = None         # BassKernelResults of the last run

_program_cache = {}
_sched_cache = {}


# ----------------------------------------------------------------- schedule
def _schedule(counts):
    """Split experts into 8*G pieces (arbitrary offsets), octets rank-matched.

    Returns (K, slots): K[g] = tile count of slot g (same on all cores);
    slots[core][g] = (expert, row_start, nrows) with nrows <= K[g]*128.
    """
    key = tuple(int(c) for c in counts)
    if key in _sched_cache:
        return _sched_cache[key]
    import heapq

    counts = [int(c) for c in counts]
    starts = np.concatenate([[0], np.cumsum(counts)]).astype(np.int64)
    live = [e for e in range(len(counts)) if counts[e] > 0]
    M = NCORES * G_SLOTS

    # LPT: split each expert into near-equal pieces, largest first
    p = {e: 1 for e in live}
    h = [(-counts[e], e) for e in live]
    heapq.heapify(h)
    while sum(p.values()) < min(M, sum(counts[e] for e in live)):
        sz, e = heapq.heappop(h)
        if p[e] >= counts[e]:
            continue
        p[e] += 1
        heapq.heappush(h, (-counts[e] / p[e], e))
    pieces = []  # [size, expert]
    for e in live:
        n = p[e]
        base, rem = divmod(counts[e], n)
        for i in range(n):
            pieces.append([base + (1 if i < rem else 0), e])
    while len(pieces) < M:
        pieces.append([0, -1])

    def T_of(ps):
        s = sorted(ps, key=lambda t: -t[0])
        return sum((s[g * NCORES][0] + P - 1) // P for g in range(G_SLOTS))

    # targeted descent: shed each octet-max piece's overhang onto siblings
    def targeted(ps):
        best = T_of(ps)
        for _ in range(60):
            order = sorted(range(len(ps)), key=lambda i: -ps[i][0])
            octet_K = {}
            for g in range(G_SLOTS):
                Kg = (ps[order[g * NCORES]][0] + P - 1) // P
                for i in order[g * NCORES:(g + 1) * NCORES]:
                    octet_K[i] = Kg
            improved = False
            for g in range(G_SLOTS):
                i = order[g * NCORES]
                sz, e = ps[i]
                if e < 0 or sz == 0:
                    continue
                overh = sz - ((sz + P - 1) // P - 1) * P
                room = sorted(
                    ((octet_K[j] * P - ps[j][0], j)
                     for j in range(len(ps))
                     if ps[j][1] == e and j != i),
                    reverse=True)
                need, moves = overh, []
                for rm, j in room:
                    if need <= 0:
                        break
                    take = min(rm, need)
                    if take > 0:
                        moves.append((j, take))
                        need -= take
                if need <= 0 and moves:
                    undo = [(j, ps[j][0]) for j, _ in moves] + [(i, sz)]
                    for j, t in moves:
                        ps[j][0] += t
                    ps[i][0] -= overh
                    nT = T_of(ps)
                    if nT < best:
                        best = nT
                        improved = True
                    else:
                        for j, v in undo:
                            ps[j][0] = v
            if not improved:
                break
        return best

    # local search: shift tokens between same-expert sibling pieces
    rng = np.random.default_rng(1)
    targeted(pieces)
    best = T_of(pieces)
    sib_idx = {}
    for j, (_, e) in enumerate(pieces):
        sib_idx.setdefault(e, []).append(j)
    for _ in range(120000):
        i = int(rng.integers(len(pieces)))
        e = pieces[i][1]
        if e < 0 or len(sib_idx[e]) < 2:
            continue
        sibs = sib_idx[e]
        j = sibs[int(rng.integers(len(sibs)))]
        if j == i:
            continue
        amt = int(rng.integers(1, 257))
        if pieces[i][0] <= amt:
            continue
        pieces[i][0] -= amt
        pieces[j][0] += amt
        c = T_of(pieces)
        if c <= best:
            best = c
        else:
            pieces[i][0] += amt
            pieces[j][0] -= amt

    # assign offsets within each expert in piece order
    off_in_e = {e: 0 for e in live}
    recs = []  # (size, expert, row_start)
    for sz, e in pieces:
        if e < 0 or sz == 0:
            recs.append((0, 0, 0))
        else:
            recs.append((sz, e, int(starts[e]) + off_in_e[e]))
            off_in_e[e] += sz
    recs.sort(key=lambda t: -t[0])

    K = []
    slots = [[] for _ in range(NCORES)]
    for g in range(G_SLOTS):
        octet = recs[g * NCORES:(g + 1) * NCORES]
        K.append((octet[0][0] + P - 1) // P)
        for i, (sz, e, r0) in enumerate(octet):
            slots[i].append((e, int(r0), int(sz)))
    while K and K[-1] == 0:
        K.pop()
        for s in slots:
            s.pop()
    _sched_cache[key] = (K, slots)
    return K, slots


def _chunks(kg, first_slot):
    """Chunk sizes for a slot of kg tiles. Slot 0 ramps up (small first
    chunk so the first matmul starts early)."""
    sizes = []
    rem = kg
    if first_slot:
        for c in (2, 6, 10):
            if rem <= 0:
                break
            c = min(c, rem)
            sizes.append(c)
            rem -= c
    while rem > 0:
        c = min(CX, rem)
        sizes.append(c)
        rem -= c
    return sizes


# ------------------------------------------------------------ device program
def _build_program(K, mode, bias_zero):
    G = len(K)
    T = sum(K)
    nc = bacc.Bacc(None)

    mmdt = {"f32r": F32R, "f32": F32, "bf16": BF16, "f8": F8E3}[mode]
    outdt = F16 if mode in ("bf16", "f8") else F32
    xt_d = nc.declare_dram_parameter("xt", [P, KT, T * P], mmdt, isOutput=False)
    wt_d = nc.declare_dram_parameter("wt", [G, P, KT, OUT_FEAT], mmdt, isOutput=False)
    if not bias_zero:
        b_d = nc.declare_dram_parameter("bias", [G, 1, OUT_FEAT], F32, isOutput=False)
    out_d = nc.declare_dram_parameter("out", [P, T, OUT_FEAT], outdt, isOutput=True)

    with tile.TileContext(nc) as tc:
        with (
            tc.tile_pool(name="w", bufs=WBUFS) as wp,
            tc.tile_pool(name="x", bufs=XBUFS) as xp,
            tc.tile_pool(name="b", bufs=2) as bp,
            tc.tile_pool(name="o", bufs=OBUFS) as op,
            tc.tile_pool(name="ps", bufs=8, space=bass.MemorySpace.PSUM) as pp,
        ):
            # PE warm-up: matmuls on scratch data at t~0 so the HAM
            # clock-gate reaches 8/8 (2.4 GHz) before the first real matmul
            # (which otherwise runs its first ~3.4us at 1.2 GHz). Reuses the
            # x and psum buffer rings (first allocation each) - no footprint.
            warm_sb = xp.tile([P, KT, CX * P], mmdt, tag="x")
            nc.vector.memset(warm_sb[:, 0, :384], 0.0)
            ps = pp.tile([P, OUT_FEAT], F32, tag="ps")
            for _ in range(36):
                nc.tensor.matmul(ps[:, :256], warm_sb[:, 0, :P],
                                 warm_sb[:, 0, 128:384],
                                 start=True, stop=True)

            off = 0
            ncopy = 0
            nstore = 0
            for g in range(G):
                kg = K[g]
                # slot 0's weights go on the sync queue, between the first
                # (small) x chunk and the rest, so the first matmuls start as
                # soon as those two transfers land; later slots prefetch on
                # the gpsimd queue
                w_sb = wp.tile([P, KT, OUT_FEAT], mmdt, tag="w")
                if g != 0:
                    nc.gpsimd.dma_start(w_sb[:], wt_d[g])
                if not bias_zero:
                    b1_sb = bp.tile([1, OUT_FEAT], F32, tag="b1")
                    (nc.scalar if g == 0 else nc.gpsimd).dma_start(
                        b1_sb[:], b_d[g])
                    b_sb = bp.tile([P, OUT_FEAT], F32, tag="b")
                    nc.gpsimd.partition_broadcast(b_sb[:], b1_sb[:])

                c0 = 0
                chunk_list = _chunks(kg, g == 0)
                for ci, cw in enumerate(chunk_list):
                    w_cols = cw * P
                    col0 = (off + c0) * P
                    x_sb = xp.tile([P, KT, CX * P], mmdt, tag="x")
                    nc.sync.dma_start(
                        x_sb[:, :, :w_cols], xt_d[:, :, col0:col0 + w_cols])
                    if g == 0 and ci == 0:
                        for k in range(KT):
                            nc.sync.dma_start(w_sb[:, k, :], wt_d[g, :, k, :])
                    o_sb = op.tile([P, CX, OUT_FEAT], outdt)
                    last_chunk = (g == G - 1 and ci == len(chunk_list) - 1)
                    step = 4 if last_chunk else (cw + 1) // 2
                    prev_s = 0
                    for t in range(cw):
                        ps = pp.tile([P, OUT_FEAT], F32, tag="ps")
                        for k in range(KT):
                            nc.tensor.matmul(
                                ps[:], x_sb[:, k, t * P:(t + 1) * P],
                                w_sb[:, k, :],
                                start=(k == 0), stop=(k == KT - 1))
                        # psum -> sbuf, rotating engines to keep any single
                        # one off the critical path
                        if bias_zero:
                            if ncopy % 3 != 2:
                                nc.vector.tensor_copy(o_sb[:, t, :], ps[:])
                            else:
                                nc.scalar.copy(o_sb[:, t, :], ps[:])
                        else:
                            nc.vector.tensor_add(o_sb[:, t, :], ps[:], b_sb[:])
                        ncopy += 1
                        # store slices as soon as their copies are done,
                        # alternating hw queues so neither stub stream nor
                        # queue becomes the straggler
                        if t - prev_s + 1 >= step or t == cw - 1:
                            nc.scalar.dma_start(
                                out_d[:, off + c0 + prev_s:off + c0 + t + 1, :],
                                o_sb[:, prev_s:t + 1, :])
                            prev_s = t + 1
                    c0 += cw
                off += kg
    nc.compile()
    return nc


# ------------------------------------------------------------------- kernel
def kernel(inp, fwd_expert_count, weight, bias):
    inp = np.asarray(inp, dtype=np.float32)
    weight = np.asarray(weight, dtype=np.float32)
    bias = np.asarray(bias, dtype=np.float32)
    counts = np.asarray(fwd_expert_count)

    K, slots = _schedule(counts)
    G, T = len(K), sum(K)
    off = np.concatenate([[0], np.cumsum(K)]).astype(np.int64)

    bias_zero = not bool(np.any(bias))
    key = (tuple(K), MODE, bias_zero)
    if key not in _program_cache:
        _program_cache[key] = _build_program(K, MODE, bias_zero)
    nc = _program_cache[key]

    npdt = {"bf16": ml_dtypes.bfloat16, "f8": ml_dtypes.float8_e3m4}.get(
        MODE, np.float32)
    xs, ws = _MODES.get(MODE, (1.0, 1.0))

    # per-expert transposed weights [P, KT, OUT]: wT[p, k, o] = weight[e][o, 128k+p]
    wT = {}
    for e in set(e for s in slots for (e, _, n) in s if n > 0):
        wT[e] = np.ascontiguousarray(
            weight[e].T.reshape(KT, P, OUT_FEAT).transpose(1, 0, 2) * ws
        ).astype(npdt)

    in_maps = []
    for core in range(NCORES):
        xt = np.zeros((P, KT, T * P), dtype=npdt)
        wt = np.zeros((G, P, KT, OUT_FEAT), dtype=npdt)
        brep = np.zeros((G, 1, OUT_FEAT), dtype=np.float32)
        for g, (e, r0, n) in enumerate(slots[core]):
            if n > 0:
                blk = inp[r0:r0 + n].T.reshape(KT, P, n).transpose(1, 0, 2)
                col0 = int(off[g]) * P
                if xs != 1.0:
                    blk = blk * xs
                xt[:, :, col0:col0 + n] = blk.astype(npdt)
                wt[g] = wT[e]
                brep[g, 0] = bias[e] * (xs * ws)
        m = {"xt": xt, "wt": wt}
        if not bias_zero:
            m["bias"] = brep
        in_maps.append(m)

    global LAST_RESULT
    res = run_bass_kernel_spmd(
        nc, in_maps, list(range(NCORES)),
        trace=TRACE, trace_cores=list(range(NCORES)) if TRACE else None,
        stitch_traces=False)
    LAST_RESULT = res

    out = np.empty((int(np.sum(np.asarray(counts, dtype=np.int64))), OUT_FEAT),
                   dtype=np.float32)
    for core in range(NCORES):
        arr = res.results[core]["out"]  # [P, T, OUT]
        for g, (e, r0, n) in enumerate(slots[core]):
            if n > 0:
                o0 = int(off[g])
                kg = K[g]
                blk = np.asarray(arr[:, o0:o0 + kg, :], dtype=np.float32)
                blk = blk.transpose(1, 0, 2).reshape(kg * P, OUT_FEAT)
                out[r0:r0 + n] = blk[:n]
    if xs * ws != 1.0:
        out *= 1.0 / (xs * ws)
    return out
